# revision 60
# baseline (speedup 1.0000x reference)
"""ConvLinformer self-attention on 8 Trainium2 NeuronCores.

Sharding: 8 cores = (batch b, sequence-half s); B=4, N=4096 -> each core owns
2048 sequence rows of one batch. The conv (the dominant 275 GFLOP op) contracts
over the sequence dim, so each core computes a partial conv over its own rows
using only its half of the conv weight (host pre-transposed to [i, k, o] layout
for clean DMA + matmul lhsT tiles); a pairwise AllReduce of the small (256,1024)
conv output completes it. Attention (all 8 heads, own rows) then needs no
further communication, and neither does the output projection.

v2: fp16 operands for all matmuls (PSUM stays fp32), AllReduce overlapped
behind the q-projection, tighter startup prefetch, and a softmax tail that
row-sums via an all-ones 128x128 stationary operand so the sums land
pre-broadcast in PSUM (vector reciprocal + multiply, no serial [1,512]
reciprocal, no gpsimd broadcast on the critical path).
"""

import sys

sys.path.insert(0, "/opt/trn_rl_repo")

import numpy as np

B, N, D = 4, 4096, 1024
H, DH, K = 8, 128, 256
KER = 32
PADL = 15
NL = N // 2          # rows per core
NCORES = 8
SCALE = DH ** -0.5

_CACHE = {}


def _build(single_core=False):
    import concourse.bacc as bacc
    import concourse.mybir as mybir
    import concourse.tile as tile

    FP32 = mybir.dt.float32
    FP32R = mybir.dt.float32r
    FP16 = mybir.dt.float16
    ACTF = mybir.ActivationFunctionType

    nc = bacc.Bacc("TRN2", target_bir_lowering=False, debug=False,
                   num_devices=1 if single_core else NCORES)

    # host pre-swizzled to [128 partitions, 8 blocks, cols] so each SBUF
    # tile loads with a single contiguous DMA (DMA triggers cost ~600ns
    # of queue time each)
    xT = nc.dram_tensor("xT", (128, 8, NL), FP16, kind="ExternalInput")
    wqT = nc.dram_tensor("wqT", (128, 8, D), FP16, kind="ExternalInput")
    wkT = nc.dram_tensor("wkT", (128, 8, D), FP16, kind="ExternalInput")
    woT = nc.dram_tensor("woT", (128, 8, D), FP16, kind="ExternalInput")
    wpkT = nc.dram_tensor("wpkT", (NL, KER * K), FP16, kind="ExternalInput")
    bpk_in = nc.dram_tensor("bpk", (128, 2), FP32, kind="ExternalInput")
    bo_in = nc.dram_tensor("bo", (1, D), FP32, kind="ExternalInput")
    ones_in = nc.dram_tensor("ones", (128, 128), FP32R, kind="ExternalInput")
    ident_in = nc.dram_tensor("ident", (128, 128), FP32R, kind="ExternalInput")
    y_out = nc.dram_tensor("y", (NL, D), FP16, kind="ExternalOutput")

    IT = NL // 128        # 16 i-tiles
    TPAD = 1056           # padded conv spatial width (15 + 1024 + 17)

    with tile.TileContext(nc) as tc:
        # ---- long-lived pools -------------------------------------------
        consts = tc.alloc_tile_pool(name="consts", bufs=1, side="left")
        p_wo = tc.alloc_tile_pool(name="wo", bufs=1, side="left")
        p_w = tc.alloc_tile_pool(name="w", bufs=1, side="left")
        p_w2 = tc.alloc_tile_pool(name="w2", bufs=1, side="left")
        p_x = tc.alloc_tile_pool(name="x", bufs=1, side="left")
        p_slab = tc.alloc_tile_pool(name="slab", bufs=4, side="left")
        p_keys = tc.alloc_tile_pool(name="keys", bufs=3, side="left")
        ps_conv = tc.alloc_tile_pool(name="convps", bufs=1, space="PSUM")
        ps_k = tc.alloc_tile_pool(name="kps", bufs=2, space="PSUM")

        # critical-path DMAs first: the first keys matmul needs wk[a] + the
        # leading x columns; interleave a-wise so matmul a=0 can start early
        # two HW DGE queues exist (SP + Activation): alternate bulk DMAs
        # across them so independent transfers run in parallel
        DQ = [nc.sync, nc.scalar]
        wkT_s = p_w.tile([128, 8, D], FP16, tag="wk")
        xT_s = p_x.tile([128, 8, NL], FP16, tag="xT")

        def emit_slab_dma(i, kh):
            slab = p_slab.tile([128, 16 * K], FP16, tag="slab")
            DQ[kh].dma_start(
                out=slab[:],
                in_=wpkT.ap()[i * 128:(i + 1) * 128,
                              kh * 16 * K:(kh + 1) * 16 * K])
            return slab

        # contiguous per-block chunks, balanced across both DGE queues in
        # keys-consumption order (a=0..7 with the matching wk pair just
        # ahead); slab0 lands mid-stream so conv i=0 is never slab-bound
        ones = consts.tile([128, 128], FP32R, tag="ones")
        nc.sync.dma_start(out=ones[:], in_=ones_in.ap())
        slabs = []
        for a in range(0, 8, 2):
            nc.scalar.dma_start(out=wkT_s[:, a:a + 2, :],
                                in_=wkT.ap()[:, a:a + 2, :])
            nc.sync.dma_start(out=xT_s[:, a, :], in_=xT.ap()[:, a, :])
            nc.scalar.dma_start(out=xT_s[:, a + 1, :], in_=xT.ap()[:, a + 1, :])
            if a == 2:
                slabs.append(emit_slab_dma(0, 0))
            if a == 4:
                slabs.append(emit_slab_dma(0, 1))

        ident = consts.tile([128, 128], FP32R, tag="ident")
        nc.sync.dma_start(out=ident[:], in_=ident_in.ap())
        bpk_t = consts.tile([128, 2], FP32, tag="bpk")
        nc.sync.dma_start(out=bpk_t[:], in_=bpk_in.ap())
        bo_row = consts.tile([1, D], FP32, tag="borow")
        nc.sync.dma_start(out=bo_row[:], in_=bo_in.ap())
        bo_bc = consts.tile([128, D], FP32, tag="bobc")
        nc.gpsimd.partition_broadcast(bo_bc[:], bo_row[:])

        # ---- P1: keys production + conv accumulation --------------------
        cps = [[ps_conv.tile([128, 512], FP32, tag=f"cps{o}{t}", name=f"cps{o}{t}")
                for t in range(2)] for o in range(2)]

        def emit_keys(i):
            pks = []
            for tch in range(2):
                psk = ps_k.tile([128, 512], FP32, tag="psk")
                for a in range(8):
                    nc.tensor.matmul(
                        psk[:], xT_s[:, a, i * 128:(i + 1) * 128],
                        wkT_s[:, a, tch * 512:(tch + 1) * 512],
                        start=(a == 0), stop=(a == 7))
                pks.append(psk)
            kt = p_keys.tile([128, TPAD], FP16, tag="keys")
            # fp16 memset via Copy(in*0.0)
            nc.scalar.activation(kt[:, 0:PADL], pks[0][:, 0:PADL],
                                 ACTF.Copy, scale=0.0)
            nc.scalar.activation(kt[:, PADL + D:TPAD],
                                 pks[1][:, 0:TPAD - PADL - D],
                                 ACTF.Copy, scale=0.0)
            nc.scalar.activation(kt[:, PADL:PADL + 512], pks[0][:], ACTF.Copy)
            nc.scalar.activation(kt[:, PADL + 512:PADL + D], pks[1][:], ACTF.Copy)
            return kt

        # prefetch tiles for later weights; DMAs are spread through P1 while
        # the DMA engines are otherwise idle
        wqT_s = p_w2.tile([128, 8, D], FP16, tag="wq")
        woT_s = p_wo.tile([128, 8, D], FP16, tag="wo")

        kt_cur = emit_keys(0)
        for i in range(IT):
            kt_next = emit_keys(i + 1) if i + 1 < IT else None
            if i + 1 < IT:
                slabs += [emit_slab_dma(i + 1, 0), emit_slab_dma(i + 1, 1)]
            if i == 1:
                nc.sync.dma_start(out=wqT_s[:], in_=wqT.ap())
            if i == 3:
                nc.sync.dma_start(out=woT_s[:], in_=woT.ap())
            for kh in range(2):
                slab = slabs.pop(0)
                for k16 in range(16):
                    k = kh * 16 + k16
                    for och in range(2):
                        lhsT = slab[:, k16 * K + och * 128:k16 * K + och * 128 + 128]
                        for tch in range(2):
                            nc.tensor.matmul(
                                cps[och][tch][:], lhsT,
                                kt_cur[:, k + tch * 512:k + tch * 512 + 512],
                                start=(i == 0 and k == 0),
                                stop=(i == IT - 1 and k == KER - 1))
            kt_cur = kt_next

        p_keys.release()
        p_slab.release()
        ps_k.release()
        # take the freed kps banks (WAR-safe: conv matmuls are their last
        # readers) rather than the conv banks still being drained below
        ps_q = tc.alloc_tile_pool(name="qps", bufs=2, space="PSUM")

        # ---- P3a: drain conv PSUM and launch the pairwise AllReduce ------
        # (emitted BEFORE the q-projection so the collective overlaps it;
        # no PE instructions in this block)
        p_qT = tc.alloc_tile_pool(name="qT", bufs=1, side="right")
        p_kcf = tc.alloc_tile_pool(name="kcf", bufs=1, side="right")
        p_kc = tc.alloc_tile_pool(name="kc", bufs=2, side="right")
        p_dram = tc.alloc_tile_pool(name="cc", bufs=1, space="DRAM")

        cc_in = p_dram.tile([2, 128, D], FP16, tag="ccin")
        cc_out = p_dram.tile([2, 128, D], FP16, tag="ccout")
        for och in range(2):
            kcp = p_kc.tile([128, D], FP16, tag="kcio")
            for tch in range(2):
                nc.scalar.activation(kcp[:, tch * 512:(tch + 1) * 512],
                                     cps[och][tch][:], ACTF.Copy)
            nc.sync.dma_start(out=cc_in[och], in_=kcp[:])
        if single_core:
            nc.sync.dma_start(out=cc_out[:], in_=cc_in[:])
        else:
            nc.gpsimd.collective_compute(
                "AllReduce", mybir.AluOpType.add,
                replica_groups=[[0, 1], [2, 3], [4, 5], [6, 7]],
                ins=[cc_in[:]], outs=[cc_out[:]])
        # conv result + bias, in fp32r for the psa/pss matmul operands
        kc_b = p_kcf.tile([128, 2, D], FP32R, tag="kcb")
        for och in range(2):
            kcs = p_kc.tile([128, D], FP16, tag="kcio")
            nc.sync.dma_start(out=kcs[:], in_=cc_out[och])
            nc.vector.tensor_scalar_add(kc_b[:, och, :], kcs[:],
                                        bpk_t[:, och:och + 1])

        # ---- P2: qT = Wq @ x^T  (t on partitions, n free; overlaps the
        # collective above) -----------------------------------------------
        qT_s = p_qT.tile([128, 8, NL], FP16, tag="qT")
        for tt in range(8):
            for nch in range(4):
                psq = ps_q.tile([128, 512], FP32, tag="psq")
                for a in range(8):
                    nc.tensor.matmul(
                        psq[:], wqT_s[:, a, tt * 128:(tt + 1) * 128],
                        xT_s[:, a, nch * 512:(nch + 1) * 512],
                        start=(a == 0), stop=(a == 7))
                nc.scalar.activation(qT_s[:, tt, nch * 512:(nch + 1) * 512],
                                     psq[:], ACTF.Copy)
        ps_q.release()
        ps_conv.release()
        p_x.release()
        p_w2.release()
        p_w.release()

        # ---- P3b: transpose conv output for the dots lhsT ----------------
        ps_t = tc.alloc_tile_pool(name="tps", bufs=2, space="PSUM")
        kcT = p_kcf.tile([128, 8, K], FP16, tag="kcT")
        for tt in range(8):
            pst = ps_t.tile([128, K], FP32R, tag="pst")
            nc.tensor.transpose(pst[:, 0:128],
                                kc_b[:, 0, tt * 128:(tt + 1) * 128], ident[:])
            nc.tensor.transpose(pst[:, 128:256],
                                kc_b[:, 1, tt * 128:(tt + 1) * 128], ident[:])
            nc.scalar.activation(kcT[:, tt, :], pst[:], ACTF.Copy)
        p_kc.release()
        ps_t.release()

        # ---- P4+P5: attention with the output projection interleaved -----
        # P5 tiles for a finished n-chunk are emitted between the next
        # chunk's attention iterations, keeping the PE fed while the
        # exp->sum->normalize chains resolve (and keeping HAM warm).
        p_attn = tc.alloc_tile_pool(name="attnT", bufs=1, side="left")
        p_y = tc.alloc_tile_pool(name="ysb", bufs=3, side="right")
        p_exp = tc.alloc_tile_pool(name="exp", bufs=6, side="right")
        p_rec = tc.alloc_tile_pool(name="rec", bufs=3, side="right")
        p_rec = tc.alloc_tile_pool(name="rec", bufs=3, side="right")
        # 256-wide iterations, manually packed into full PSUM banks:
        # one [128,512] bank per iteration's psd lo+hi (4 banks = depth 4),
        # one bank holding two iterations' pss halves, one for psa halves,
        # two banks for the interleaved output projection -> 8 total
        ps_d = tc.alloc_tile_pool(name="dps", bufs=4, space="PSUM")
        ps_s = tc.alloc_tile_pool(name="sps", bufs=1, space="PSUM")
        ps_a = tc.alloc_tile_pool(name="aps", bufs=1, space="PSUM")
        ps_y = tc.alloc_tile_pool(name="yps", bufs=2, space="PSUM")

        psd_banks = [ps_d.tile([128, 512], FP32, tag="psd", name=f"psdb{i}")
                     for i in range(4)]
        pss_bank = ps_s.tile([128, 512], FP32, tag="pss", name="pssb")
        psa_bank = ps_a.tile([128, 512], FP32, tag="psa", name="psab")

        attn_outT = p_attn.tile([128, 8, NL], FP16, tag="attnT")

        def emit_dots(j, nch, h):
            nsl = slice(nch * 256, (nch + 1) * 256)
            psd_lo = psd_banks[j % 4][:, 0:256]
            nc.tensor.matmul(psd_lo, kcT[:, h, 0:128], qT_s[:, h, nsl],
                             start=True, stop=True)
            psd_hi = psd_banks[j % 4][:, 256:512]
            nc.tensor.matmul(psd_hi, kcT[:, h, 128:256], qT_s[:, h, nsl],
                             start=True, stop=True)
            # lo/hi are adjacent halves of one bank: a single exp covers both
            e = p_exp.tile([128, 512], FP32R, tag="exp", name=f"e{nch}_{h}")
            nc.scalar.activation(e[:], psd_banks[j % 4][:], ACTF.Exp,
                                 scale=SCALE)
            return nsl, e

        def emit_tail(j, nch, h, st):
            nsl, e = st
            e_lo, e_hi = e[:, 0:256], e[:, 256:512]
            half = slice((j % 2) * 256, (j % 2) * 256 + 256)
            # all-ones stationary -> every PSUM partition holds the K-sum:
            # the normalizer arrives pre-broadcast
            pss = pss_bank[:, half]
            nc.tensor.matmul(pss, ones[:], e_lo, start=True, stop=False)
            nc.tensor.matmul(pss, ones[:], e_hi, start=False, stop=True)
            psa = psa_bank[:, half]
            nc.tensor.matmul(psa, kc_b[:, 0, h * 128:(h + 1) * 128],
                             e_lo, start=True, stop=False)
            nc.tensor.matmul(psa, kc_b[:, 1, h * 128:(h + 1) * 128],
                             e_hi, start=False, stop=True)
            rec = p_rec.tile([128, 256], FP32, tag="rec", name=f"rec{nch}_{h}")
            nc.vector.reciprocal_approx_fast(rec[:], pss)
            nc.vector.tensor_mul(attn_outT[:, h, nsl], psa, rec[:])

        def emit_y_tile(nt):
            psy = [ps_y.tile([128, 512], FP32, tag="psy", name=f"psy{nt}_{_i}")
                   for _i in range(2)]
            for tt in range(8):
                for cch in range(2):
                    nc.tensor.matmul(
                        psy[cch][:], attn_outT[:, tt, nt * 128:(nt + 1) * 128],
                        woT_s[:, tt, cch * 512:(cch + 1) * 512],
                        start=(tt == 0), stop=(tt == 7))
            ysb = p_y.tile([128, D], FP16, tag="ysb")
            for cch in range(2):
                nc.vector.tensor_add(ysb[:, cch * 512:(cch + 1) * 512],
                                     psy[cch][:],
                                     bo_bc[:, cch * 512:(cch + 1) * 512])
            DQ[nt % 2].dma_start(out=y_out.ap()[nt * 128:(nt + 1) * 128, :],
                                 in_=ysb[:])

        seq = [(nch, h) for nch in range(8) for h in range(8)]
        pending = []          # y tiles whose attention chunk is complete
        st = emit_dots(0, *seq[0])
        for j, (nch, h) in enumerate(seq):
            nxt = emit_dots(j + 1, *seq[j + 1]) if j + 1 < len(seq) else None
            emit_tail(j, nch, h, st)
            st = nxt
            if h % 4 == 3 and pending:
                emit_y_tile(pending.pop(0))
            if h == 7:
                pending += [nch * 2 + t for t in range(2)]
        for nt in pending:
            emit_y_tile(nt)
        p_rec.release()
        p_rec.release()
        p_exp.release()
        p_y.release()
        p_kcf.release()
        p_qT.release()
        ps_y.release()
        ps_a.release()
        ps_s.release()
        ps_d.release()
        p_attn.release()
        p_wo.release()
        consts.release()

    nc.compile()
    return nc


def _get_nc():
    if "nc" not in _CACHE:
        _CACHE["nc"] = _build()
    return _CACHE["nc"]


def _swiz(wT):
    # [D, cols] -> [128, 8, cols]: partition-major blocks of 128 rows
    return np.ascontiguousarray(
        wT.reshape(8, 128, wT.shape[1]).transpose(1, 0, 2))


def _prep_inputs(x, Wq, Wk, Wpk, bpk, Wo, bo):
    wqT = _swiz(Wq.T.astype(np.float16))
    wkT = _swiz(Wk.T.astype(np.float16))
    woT = _swiz(Wo.T.astype(np.float16))
    # Wpk (K=256, N=4096, KER=32) -> [i, k, o] contiguous
    wpkT = np.ascontiguousarray(
        Wpk.astype(np.float16).transpose(1, 2, 0)).reshape(N, KER * K)
    bpk2 = np.ascontiguousarray(bpk.astype(np.float32).reshape(2, 128).T)
    bo2 = np.ascontiguousarray(bo.astype(np.float32).reshape(1, D))
    ones = np.ones((128, 128), dtype=np.float32)
    ident = np.eye(128, dtype=np.float32)
    in_maps = []
    for c in range(NCORES):
        b, s = c // 2, c % 2
        in_maps.append({
            "xT": _swiz(np.ascontiguousarray(
                x[b, s * NL:(s + 1) * NL, :].T.astype(np.float16))),
            "wqT": wqT, "wkT": wkT, "woT": woT,
            "wpkT": np.ascontiguousarray(wpkT[s * NL:(s + 1) * NL]),
            "bpk": bpk2, "bo": bo2, "ones": ones, "ident": ident,
        })
    return in_maps


def kernel(x, Wq, Wk, Wpk, bpk, Wo, bo, _trace=False, _trace_kwargs=None):
    from concourse.bass_utils import run_bass_kernel_spmd

    nc = _get_nc()
    in_maps = _prep_inputs(np.asarray(x), np.asarray(Wq), np.asarray(Wk),
                           np.asarray(Wpk), np.asarray(bpk), np.asarray(Wo),
                           np.asarray(bo))
    res = run_bass_kernel_spmd(nc, in_maps, core_ids=list(range(NCORES)),
                               trace=_trace, **(_trace_kwargs or {}))
    _CACHE["last_result"] = res
    out = np.empty((B, N, D), dtype=np.float32)
    for c in range(NCORES):
        b, s = c // 2, c % 2
        out[b, s * NL:(s + 1) * NL, :] = res.results[c]["y"].astype(np.float32)
    return out


# revision 61
# speedup vs baseline: 1.1185x; 1.1185x over previous
"""ConvLinformer self-attention on 8 Trainium2 NeuronCores.

Sharding: 8 cores = (batch b, sequence-half s); B=4, N=4096 -> each core owns
2048 sequence rows of one batch. The conv (the dominant 275 GFLOP op) contracts
over the sequence dim, so each core computes a partial conv over its own rows
using only its half of the conv weight (host pre-transposed to [i, k, o] layout
for clean DMA + matmul lhsT tiles); a pairwise AllReduce of the small (256,1024)
conv output completes it. Attention (all 8 heads, own rows) then needs no
further communication, and neither does the output projection.

v2: fp16 operands for all matmuls (PSUM stays fp32), AllReduce overlapped
behind the q-projection, tighter startup prefetch, and a softmax tail that
row-sums via an all-ones 128x128 stationary operand so the sums land
pre-broadcast in PSUM (vector reciprocal + multiply, no serial [1,512]
reciprocal, no gpsimd broadcast on the critical path).
"""

import sys

sys.path.insert(0, "/opt/trn_rl_repo")

import numpy as np

B, N, D = 4, 4096, 1024
H, DH, K = 8, 128, 256
KER = 32
PADL = 15
NL = N // 2          # rows per core
NCORES = 8
SCALE = DH ** -0.5

_CACHE = {}


def _build(single_core=False):
    import concourse.bacc as bacc
    import concourse.mybir as mybir
    import concourse.tile as tile

    FP32 = mybir.dt.float32
    FP32R = mybir.dt.float32r
    FP16 = mybir.dt.float16
    ACTF = mybir.ActivationFunctionType

    nc = bacc.Bacc("TRN2", target_bir_lowering=False, debug=False,
                   num_devices=1 if single_core else NCORES)

    # host pre-swizzled to [128 partitions, 8 blocks, cols] so each SBUF
    # tile loads with a single contiguous DMA (DMA triggers cost ~600ns
    # of queue time each)
    xT = nc.dram_tensor("xT", (128, 8, NL), FP16, kind="ExternalInput")
    wqT = nc.dram_tensor("wqT", (128, 8, D), FP16, kind="ExternalInput")
    wkT = nc.dram_tensor("wkT", (128, 8, D), FP16, kind="ExternalInput")
    woT = nc.dram_tensor("woT", (128, 8, D), FP16, kind="ExternalInput")
    wpkT = nc.dram_tensor("wpkT", (NL, KER * K), FP16, kind="ExternalInput")
    bpk_in = nc.dram_tensor("bpk", (128, 2), FP32, kind="ExternalInput")
    bo_in = nc.dram_tensor("bo", (1, D), FP32, kind="ExternalInput")
    ones_in = nc.dram_tensor("ones", (128, 128), FP32R, kind="ExternalInput")
    ident_in = nc.dram_tensor("ident", (128, 128), FP32R, kind="ExternalInput")
    y_out = nc.dram_tensor("y", (NL, D), FP16, kind="ExternalOutput")

    IT = NL // 128        # 16 i-tiles
    TPAD = 1056           # padded conv spatial width (15 + 1024 + 17)

    with tile.TileContext(nc) as tc:
        # ---- long-lived pools -------------------------------------------
        consts = tc.alloc_tile_pool(name="consts", bufs=1, side="left")
        p_wo = tc.alloc_tile_pool(name="wo", bufs=1, side="left")
        p_w = tc.alloc_tile_pool(name="w", bufs=1, side="left")
        p_w2 = tc.alloc_tile_pool(name="w2", bufs=1, side="left")
        p_x = tc.alloc_tile_pool(name="x", bufs=1, side="left")
        p_slab = tc.alloc_tile_pool(name="slab", bufs=4, side="left")
        p_keys = tc.alloc_tile_pool(name="keys", bufs=3, side="left")
        ps_conv = tc.alloc_tile_pool(name="convps", bufs=1, space="PSUM")
        ps_k = tc.alloc_tile_pool(name="kps", bufs=2, space="PSUM")

        # critical-path DMAs first: the first keys matmul needs wk[a] + the
        # leading x columns; interleave a-wise so matmul a=0 can start early
        # two HW DGE queues exist (SP + Activation): alternate bulk DMAs
        # across them so independent transfers run in parallel
        DQ = [nc.sync, nc.scalar]
        wkT_s = p_w.tile([128, 8, D], FP16, tag="wk")
        xT_s = p_x.tile([128, 8, NL], FP16, tag="xT")

        def emit_slab_dma(i, kh):
            slab = p_slab.tile([128, 16 * K], FP16, tag="slab")
            DQ[kh].dma_start(
                out=slab[:],
                in_=wpkT.ap()[i * 128:(i + 1) * 128,
                              kh * 16 * K:(kh + 1) * 16 * K])
            return slab

        # contiguous per-block chunks, balanced across both DGE queues in
        # keys-consumption order (a=0..7 with the matching wk pair just
        # ahead); slab0 lands mid-stream so conv i=0 is never slab-bound
        ones = consts.tile([128, 128], FP32R, tag="ones")
        nc.sync.dma_start(out=ones[:], in_=ones_in.ap())
        slabs = []
        for a in range(0, 8, 2):
            nc.scalar.dma_start(out=wkT_s[:, a:a + 2, :],
                                in_=wkT.ap()[:, a:a + 2, :])
            nc.sync.dma_start(out=xT_s[:, a, :], in_=xT.ap()[:, a, :])
            nc.scalar.dma_start(out=xT_s[:, a + 1, :], in_=xT.ap()[:, a + 1, :])
            if a == 2:
                slabs.append(emit_slab_dma(0, 0))
            if a == 4:
                slabs.append(emit_slab_dma(0, 1))

        ident = consts.tile([128, 128], FP32R, tag="ident")
        nc.sync.dma_start(out=ident[:], in_=ident_in.ap())
        bpk_t = consts.tile([128, 2], FP32, tag="bpk")
        nc.sync.dma_start(out=bpk_t[:], in_=bpk_in.ap())
        bo_row = consts.tile([1, D], FP32, tag="borow")
        nc.sync.dma_start(out=bo_row[:], in_=bo_in.ap())
        bo_bc = consts.tile([128, D], FP32, tag="bobc")
        nc.gpsimd.partition_broadcast(bo_bc[:], bo_row[:])

        # ---- P1: keys production + conv accumulation --------------------
        cps = [[ps_conv.tile([128, 512], FP32, tag=f"cps{o}{t}", name=f"cps{o}{t}")
                for t in range(2)] for o in range(2)]

        def emit_keys(i):
            pks = []
            for tch in range(2):
                psk = ps_k.tile([128, 512], FP32, tag="psk")
                for a in range(8):
                    nc.tensor.matmul(
                        psk[:], xT_s[:, a, i * 128:(i + 1) * 128],
                        wkT_s[:, a, tch * 512:(tch + 1) * 512],
                        start=(a == 0), stop=(a == 7))
                pks.append(psk)
            kt = p_keys.tile([128, TPAD], FP16, tag="keys")
            # fp16 memset via Copy(in*0.0)
            nc.scalar.activation(kt[:, 0:PADL], pks[0][:, 0:PADL],
                                 ACTF.Copy, scale=0.0)
            nc.scalar.activation(kt[:, PADL + D:TPAD],
                                 pks[1][:, 0:TPAD - PADL - D],
                                 ACTF.Copy, scale=0.0)
            nc.scalar.activation(kt[:, PADL:PADL + 512], pks[0][:], ACTF.Copy)
            nc.scalar.activation(kt[:, PADL + 512:PADL + D], pks[1][:], ACTF.Copy)
            return kt

        # prefetch tiles for later weights; DMAs are spread through P1 while
        # the DMA engines are otherwise idle
        wqT_s = p_w2.tile([128, 8, D], FP16, tag="wq")
        woT_s = p_wo.tile([128, 8, D], FP16, tag="wo")

        kt_cur = emit_keys(0)
        for i in range(IT):
            kt_next = emit_keys(i + 1) if i + 1 < IT else None
            if i + 1 < IT:
                slabs += [emit_slab_dma(i + 1, 0), emit_slab_dma(i + 1, 1)]
            if i == 1:
                nc.sync.dma_start(out=wqT_s[:], in_=wqT.ap())
            if i == 3:
                nc.sync.dma_start(out=woT_s[:], in_=woT.ap())
            for kh in range(2):
                slab = slabs.pop(0)
                for k16 in range(16):
                    k = kh * 16 + k16
                    for och in range(2):
                        lhsT = slab[:, k16 * K + och * 128:k16 * K + och * 128 + 128]
                        for tch in range(2):
                            nc.tensor.matmul(
                                cps[och][tch][:], lhsT,
                                kt_cur[:, k + tch * 512:k + tch * 512 + 512],
                                start=(i == 0 and k == 0),
                                stop=(i == IT - 1 and k == KER - 1))
            kt_cur = kt_next

        p_keys.release()
        p_slab.release()
        ps_k.release()
        # take the freed kps banks (WAR-safe: conv matmuls are their last
        # readers) rather than the conv banks still being drained below
        ps_q = tc.alloc_tile_pool(name="qps", bufs=2, space="PSUM")

        # ---- P3a: drain conv PSUM and launch the pairwise AllReduce ------
        # (emitted BEFORE the q-projection so the collective overlaps it;
        # no PE instructions in this block)
        p_qT = tc.alloc_tile_pool(name="qT", bufs=1, side="right")
        p_kcf = tc.alloc_tile_pool(name="kcf", bufs=1, side="right")
        p_kc = tc.alloc_tile_pool(name="kc", bufs=2, side="right")
        p_dram = tc.alloc_tile_pool(name="cc", bufs=1, space="DRAM")

        cc_in = p_dram.tile([2, 128, D], FP16, tag="ccin")
        cc_out = p_dram.tile([2, 128, D], FP16, tag="ccout")
        for och in range(2):
            kcp = p_kc.tile([128, D], FP16, tag="kcio")
            for tch in range(2):
                nc.scalar.activation(kcp[:, tch * 512:(tch + 1) * 512],
                                     cps[och][tch][:], ACTF.Copy)
            nc.sync.dma_start(out=cc_in[och], in_=kcp[:])
        if single_core:
            nc.sync.dma_start(out=cc_out[:], in_=cc_in[:])
        else:
            nc.gpsimd.collective_compute(
                "AllReduce", mybir.AluOpType.add,
                replica_groups=[[0, 1], [2, 3], [4, 5], [6, 7]],
                ins=[cc_in[:]], outs=[cc_out[:]])
        # conv result + bias, in fp32r for the psa/pss matmul operands
        kc_b = p_kcf.tile([128, 2, D], FP32R, tag="kcb")
        for och in range(2):
            kcs = p_kc.tile([128, D], FP16, tag="kcio")
            nc.sync.dma_start(out=kcs[:], in_=cc_out[och])
            nc.vector.tensor_scalar_add(kc_b[:, och, :], kcs[:],
                                        bpk_t[:, och:och + 1])

        # ---- P2: qT = Wq @ x^T  (t on partitions, n free; overlaps the
        # collective above) -----------------------------------------------
        qT_s = p_qT.tile([128, 8, NL], FP16, tag="qT")
        for tt in range(8):
            for nch in range(4):
                psq = ps_q.tile([128, 512], FP32, tag="psq")
                for a in range(8):
                    nc.tensor.matmul(
                        psq[:], wqT_s[:, a, tt * 128:(tt + 1) * 128],
                        xT_s[:, a, nch * 512:(nch + 1) * 512],
                        start=(a == 0), stop=(a == 7))
                nc.scalar.activation(qT_s[:, tt, nch * 512:(nch + 1) * 512],
                                     psq[:], ACTF.Copy)
        ps_q.release()
        ps_conv.release()
        p_x.release()
        p_w2.release()
        p_w.release()

        # ---- P3b: transpose conv output for the dots lhsT ----------------
        ps_t = tc.alloc_tile_pool(name="tps", bufs=2, space="PSUM")
        kcT = p_kcf.tile([128, 8, K], FP16, tag="kcT")
        for tt in range(8):
            pst = ps_t.tile([128, K], FP32R, tag="pst")
            nc.tensor.transpose(pst[:, 0:128],
                                kc_b[:, 0, tt * 128:(tt + 1) * 128], ident[:])
            nc.tensor.transpose(pst[:, 128:256],
                                kc_b[:, 1, tt * 128:(tt + 1) * 128], ident[:])
            nc.scalar.activation(kcT[:, tt, :], pst[:], ACTF.Copy)
        p_kc.release()
        ps_t.release()

        # ---- P4+P5: attention with the output projection interleaved -----
        # P5 tiles for a finished n-chunk are emitted between the next
        # chunk's attention iterations, keeping the PE fed while the
        # exp->sum->normalize chains resolve (and keeping HAM warm).
        p_attn = tc.alloc_tile_pool(name="attnT", bufs=1, side="left")
        p_y = tc.alloc_tile_pool(name="ysb", bufs=3, side="right")
        p_exp = tc.alloc_tile_pool(name="exp", bufs=6, side="right")
        p_rec = tc.alloc_tile_pool(name="rec", bufs=3, side="right")
        p_rec = tc.alloc_tile_pool(name="rec", bufs=3, side="right")
        # 256-wide iterations, manually packed into full PSUM banks:
        # one [128,512] bank per iteration's psd lo+hi (4 banks = depth 4),
        # one bank holding two iterations' pss halves, one for psa halves,
        # two banks for the interleaved output projection -> 8 total
        ps_d = tc.alloc_tile_pool(name="dps", bufs=4, space="PSUM")
        ps_s = tc.alloc_tile_pool(name="sps", bufs=1, space="PSUM")
        ps_a = tc.alloc_tile_pool(name="aps", bufs=1, space="PSUM")
        ps_y = tc.alloc_tile_pool(name="yps", bufs=2, space="PSUM")

        psd_banks = [ps_d.tile([128, 512], FP32, tag="psd", name=f"psdb{i}")
                     for i in range(4)]
        pss_bank = ps_s.tile([128, 512], FP32, tag="pss", name="pssb")
        psa_bank = ps_a.tile([128, 512], FP32, tag="psa", name="psab")

        attn_outT = p_attn.tile([128, 8, NL], FP16, tag="attnT")

        def emit_dots(j, nch, h):
            # 512-wide: 213ns matmuls leave slack for the 104ns LDWEIGHTS
            # to hide (256-wide left none); psd depth 2 across the four
            # banks, interleaved y-tiles provide the chain slack
            nsl = slice(nch * 512, (nch + 1) * 512)
            psd_lo = psd_banks[(2 * j) % 4]
            nc.tensor.matmul(psd_lo[:], kcT[:, h, 0:128], qT_s[:, h, nsl],
                             start=True, stop=True)
            psd_hi = psd_banks[(2 * j + 1) % 4]
            nc.tensor.matmul(psd_hi[:], kcT[:, h, 128:256], qT_s[:, h, nsl],
                             start=True, stop=True)
            e_lo = p_exp.tile([128, 512], FP32R, tag="exp", name=f"el{nch}_{h}")
            nc.scalar.activation(e_lo[:], psd_lo[:], ACTF.Exp, scale=SCALE)
            e_hi = p_exp.tile([128, 512], FP32R, tag="exp", name=f"eh{nch}_{h}")
            nc.scalar.activation(e_hi[:], psd_hi[:], ACTF.Exp, scale=SCALE)
            return nsl, e_lo, e_hi

        def emit_tail(j, nch, h, st):
            nsl, e_lo, e_hi = st
            # all-ones stationary -> every PSUM partition holds the K-sum:
            # the normalizer arrives pre-broadcast
            pss = pss_bank[:]
            nc.tensor.matmul(pss, ones[:], e_lo[:], start=True, stop=False)
            nc.tensor.matmul(pss, ones[:], e_hi[:], start=False, stop=True)
            psa = psa_bank[:]
            nc.tensor.matmul(psa, kc_b[:, 0, h * 128:(h + 1) * 128],
                             e_lo[:], start=True, stop=False)
            nc.tensor.matmul(psa, kc_b[:, 1, h * 128:(h + 1) * 128],
                             e_hi[:], start=False, stop=True)
            rec = p_rec.tile([128, 512], FP32, tag="rec", name=f"rec{nch}_{h}")
            nc.vector.reciprocal_approx_fast(rec[:], pss)
            nc.vector.tensor_mul(attn_outT[:, h, nsl], psa, rec[:])

        def emit_y_tile(nt):
            psy = [ps_y.tile([128, 512], FP32, tag="psy", name=f"psy{nt}_{_i}")
                   for _i in range(2)]
            for tt in range(8):
                for cch in range(2):
                    nc.tensor.matmul(
                        psy[cch][:], attn_outT[:, tt, nt * 128:(nt + 1) * 128],
                        woT_s[:, tt, cch * 512:(cch + 1) * 512],
                        start=(tt == 0), stop=(tt == 7))
            ysb = p_y.tile([128, D], FP16, tag="ysb")
            for cch in range(2):
                nc.vector.tensor_add(ysb[:, cch * 512:(cch + 1) * 512],
                                     psy[cch][:],
                                     bo_bc[:, cch * 512:(cch + 1) * 512])
            DQ[nt % 2].dma_start(out=y_out.ap()[nt * 128:(nt + 1) * 128, :],
                                 in_=ysb[:])

        seq = [(nch, h) for nch in range(4) for h in range(8)]
        pending = []          # y tiles whose attention chunk is complete
        st = emit_dots(0, *seq[0])
        for j, (nch, h) in enumerate(seq):
            nxt = emit_dots(j + 1, *seq[j + 1]) if j + 1 < len(seq) else None
            emit_tail(j, nch, h, st)
            st = nxt
            if h % 2 == 1 and pending:
                emit_y_tile(pending.pop(0))
            if h == 7:
                pending += [nch * 4 + t for t in range(4)]
        for nt in pending:
            emit_y_tile(nt)
        p_rec.release()
        p_rec.release()
        p_exp.release()
        p_y.release()
        p_kcf.release()
        p_qT.release()
        ps_y.release()
        ps_a.release()
        ps_s.release()
        ps_d.release()
        p_attn.release()
        p_wo.release()
        consts.release()

    nc.compile()
    return nc


def _get_nc():
    if "nc" not in _CACHE:
        _CACHE["nc"] = _build()
    return _CACHE["nc"]


def _swiz(wT):
    # [D, cols] -> [128, 8, cols]: partition-major blocks of 128 rows
    return np.ascontiguousarray(
        wT.reshape(8, 128, wT.shape[1]).transpose(1, 0, 2))


def _prep_inputs(x, Wq, Wk, Wpk, bpk, Wo, bo):
    wqT = _swiz(Wq.T.astype(np.float16))
    wkT = _swiz(Wk.T.astype(np.float16))
    woT = _swiz(Wo.T.astype(np.float16))
    # Wpk (K=256, N=4096, KER=32) -> [i, k, o] contiguous
    wpkT = np.ascontiguousarray(
        Wpk.astype(np.float16).transpose(1, 2, 0)).reshape(N, KER * K)
    bpk2 = np.ascontiguousarray(bpk.astype(np.float32).reshape(2, 128).T)
    bo2 = np.ascontiguousarray(bo.astype(np.float32).reshape(1, D))
    ones = np.ones((128, 128), dtype=np.float32)
    ident = np.eye(128, dtype=np.float32)
    in_maps = []
    for c in range(NCORES):
        b, s = c // 2, c % 2
        in_maps.append({
            "xT": _swiz(np.ascontiguousarray(
                x[b, s * NL:(s + 1) * NL, :].T.astype(np.float16))),
            "wqT": wqT, "wkT": wkT, "woT": woT,
            "wpkT": np.ascontiguousarray(wpkT[s * NL:(s + 1) * NL]),
            "bpk": bpk2, "bo": bo2, "ones": ones, "ident": ident,
        })
    return in_maps


def kernel(x, Wq, Wk, Wpk, bpk, Wo, bo, _trace=False, _trace_kwargs=None):
    from concourse.bass_utils import run_bass_kernel_spmd

    nc = _get_nc()
    in_maps = _prep_inputs(np.asarray(x), np.asarray(Wq), np.asarray(Wk),
                           np.asarray(Wpk), np.asarray(bpk), np.asarray(Wo),
                           np.asarray(bo))
    res = run_bass_kernel_spmd(nc, in_maps, core_ids=list(range(NCORES)),
                               trace=_trace, **(_trace_kwargs or {}))
    _CACHE["last_result"] = res
    out = np.empty((B, N, D), dtype=np.float32)
    for c in range(NCORES):
        b, s = c // 2, c % 2
        out[b, s * NL:(s + 1) * NL, :] = res.results[c]["y"].astype(np.float32)
    return out


# revision 63
# speedup vs baseline: 1.1430x; 1.0219x over previous
"""ConvLinformer self-attention on 8 Trainium2 NeuronCores.

Sharding: 8 cores = (batch b, sequence-half s); B=4, N=4096 -> each core owns
2048 sequence rows of one batch. The conv (the dominant 275 GFLOP op) contracts
over the sequence dim, so each core computes a partial conv over its own rows
using only its half of the conv weight (host pre-transposed to [i, k, o] layout
for clean DMA + matmul lhsT tiles); a pairwise AllReduce of the small (256,1024)
conv output completes it. Attention (all 8 heads, own rows) then needs no
further communication, and neither does the output projection.

v2: fp16 operands for all matmuls (PSUM stays fp32), AllReduce overlapped
behind the q-projection, tighter startup prefetch, and a softmax tail that
row-sums via an all-ones 128x128 stationary operand so the sums land
pre-broadcast in PSUM (vector reciprocal + multiply, no serial [1,512]
reciprocal, no gpsimd broadcast on the critical path).
"""

import sys

sys.path.insert(0, "/opt/trn_rl_repo")

import numpy as np

B, N, D = 4, 4096, 1024
H, DH, K = 8, 128, 256
KER = 32
PADL = 15
NL = N // 2          # rows per core
NCORES = 8
SCALE = DH ** -0.5

_CACHE = {}


def _build(single_core=False):
    import concourse.bacc as bacc
    import concourse.mybir as mybir
    import concourse.tile as tile

    FP32 = mybir.dt.float32
    FP32R = mybir.dt.float32r
    FP16 = mybir.dt.float16
    ACTF = mybir.ActivationFunctionType

    nc = bacc.Bacc("TRN2", target_bir_lowering=False, debug=False,
                   num_devices=1 if single_core else NCORES)

    # host pre-swizzled to [128 partitions, 8 blocks, cols] so each SBUF
    # tile loads with a single contiguous DMA (DMA triggers cost ~600ns
    # of queue time each)
    # chunk-major x: [part, 512-col chunk, block, col] so each chunk is one
    # contiguous DMA on both sides and the first chunk feeds keys tiles 0-3
    xT = nc.dram_tensor("xT", (128, 4, 8, 512), FP16, kind="ExternalInput")
    wqT = nc.dram_tensor("wqT", (128, 8, D), FP16, kind="ExternalInput")
    wkT = nc.dram_tensor("wkT", (128, 8, D), FP16, kind="ExternalInput")
    woT = nc.dram_tensor("woT", (128, 8, D), FP16, kind="ExternalInput")
    wpkT = nc.dram_tensor("wpkT", (NL, KER * K), FP16, kind="ExternalInput")
    bpk_in = nc.dram_tensor("bpk", (128, 2), FP32, kind="ExternalInput")
    bo_in = nc.dram_tensor("bo", (1, D), FP32, kind="ExternalInput")
    ones_in = nc.dram_tensor("ones", (128, 128), FP32R, kind="ExternalInput")
    ident_in = nc.dram_tensor("ident", (128, 128), FP32R, kind="ExternalInput")
    y_out = nc.dram_tensor("y", (NL, D), FP16, kind="ExternalOutput")

    IT = NL // 128        # 16 i-tiles
    TPAD = 1056           # padded conv spatial width (15 + 1024 + 17)

    with tile.TileContext(nc) as tc:
        # ---- long-lived pools -------------------------------------------
        consts = tc.alloc_tile_pool(name="consts", bufs=1, side="left")
        p_wo = tc.alloc_tile_pool(name="wo", bufs=1, side="left")
        p_w = tc.alloc_tile_pool(name="w", bufs=1, side="left")
        p_w2 = tc.alloc_tile_pool(name="w2", bufs=1, side="left")
        p_x = tc.alloc_tile_pool(name="x", bufs=1, side="left")
        p_slab = tc.alloc_tile_pool(name="slab", bufs=4, side="left")
        p_keys = tc.alloc_tile_pool(name="keys", bufs=3, side="left")
        ps_conv = tc.alloc_tile_pool(name="convps", bufs=1, space="PSUM")
        ps_k = tc.alloc_tile_pool(name="kps", bufs=2, space="PSUM")

        # critical-path DMAs first: the first keys matmul needs wk[a] + the
        # leading x columns; interleave a-wise so matmul a=0 can start early
        # two HW DGE queues exist (SP + Activation): alternate bulk DMAs
        # across them so independent transfers run in parallel
        DQ = [nc.sync, nc.scalar]
        wkT_s = p_w.tile([128, 8, D], FP16, tag="wk")
        xT_s = p_x.tile([128, 4, 8, 512], FP16, tag="xT")

        def emit_slab_dma(i, kh):
            slab = p_slab.tile([128, 16 * K], FP16, tag="slab")
            DQ[kh].dma_start(
                out=slab[:],
                in_=wpkT.ap()[i * 128:(i + 1) * 128,
                              kh * 16 * K:(kh + 1) * 16 * K])
            return slab

        # contiguous per-block chunks, balanced across both DGE queues in
        # keys-consumption order (a=0..7 with the matching wk pair just
        # ahead); slab0 lands mid-stream so conv i=0 is never slab-bound
        ones = consts.tile([128, 128], FP32R, tag="ones")
        nc.sync.dma_start(out=ones[:], in_=ones_in.ap())
        nc.sync.dma_start(out=xT_s[:, 0], in_=xT.ap()[:, 0])
        for a in range(0, 8, 2):
            nc.scalar.dma_start(out=wkT_s[:, a:a + 2, :],
                                in_=wkT.ap()[:, a:a + 2, :])
        slabs = [emit_slab_dma(0, 0), emit_slab_dma(0, 1)]
        nc.sync.dma_start(out=xT_s[:, 1], in_=xT.ap()[:, 1])
        nc.scalar.dma_start(out=xT_s[:, 2], in_=xT.ap()[:, 2])
        nc.sync.dma_start(out=xT_s[:, 3], in_=xT.ap()[:, 3])

        ident = consts.tile([128, 128], FP32R, tag="ident")
        nc.sync.dma_start(out=ident[:], in_=ident_in.ap())
        bpk_t = consts.tile([128, 2], FP32, tag="bpk")
        nc.sync.dma_start(out=bpk_t[:], in_=bpk_in.ap())
        bo_row = consts.tile([1, D], FP32, tag="borow")
        nc.sync.dma_start(out=bo_row[:], in_=bo_in.ap())
        bo_bc = consts.tile([128, D], FP32, tag="bobc")
        nc.gpsimd.partition_broadcast(bo_bc[:], bo_row[:])

        # ---- P1: keys production + conv accumulation --------------------
        cps = [[ps_conv.tile([128, 512], FP32, tag=f"cps{o}{t}", name=f"cps{o}{t}")
                for t in range(2)] for o in range(2)]

        def emit_keys(i):
            pks = []
            for tch in range(2):
                psk = ps_k.tile([128, 512], FP32, tag="psk")
                for a in range(8):
                    nc.tensor.matmul(
                        psk[:], xT_s[:, i // 4, a,
                                     (i % 4) * 128:(i % 4) * 128 + 128],
                        wkT_s[:, a, tch * 512:(tch + 1) * 512],
                        start=(a == 0), stop=(a == 7))
                pks.append(psk)
            kt = p_keys.tile([128, TPAD], FP16, tag="keys")
            # fp16 memset via Copy(in*0.0)
            nc.scalar.activation(kt[:, 0:PADL], pks[0][:, 0:PADL],
                                 ACTF.Copy, scale=0.0)
            nc.scalar.activation(kt[:, PADL + D:TPAD],
                                 pks[1][:, 0:TPAD - PADL - D],
                                 ACTF.Copy, scale=0.0)
            nc.scalar.activation(kt[:, PADL:PADL + 512], pks[0][:], ACTF.Copy)
            nc.scalar.activation(kt[:, PADL + 512:PADL + D], pks[1][:], ACTF.Copy)
            return kt

        # prefetch tiles for later weights; DMAs are spread through P1 while
        # the DMA engines are otherwise idle
        wqT_s = p_w2.tile([128, 8, D], FP16, tag="wq")
        woT_s = p_wo.tile([128, 8, D], FP16, tag="wo")

        kt_cur = emit_keys(0)
        for i in range(IT):
            kt_next = emit_keys(i + 1) if i + 1 < IT else None
            if i + 1 < IT:
                slabs += [emit_slab_dma(i + 1, 0), emit_slab_dma(i + 1, 1)]
            if i == 1:
                nc.sync.dma_start(out=wqT_s[:], in_=wqT.ap())
            if i == 3:
                nc.sync.dma_start(out=woT_s[:], in_=woT.ap())
            for kh in range(2):
                slab = slabs.pop(0)
                for k16 in range(16):
                    k = kh * 16 + k16
                    for och in range(2):
                        lhsT = slab[:, k16 * K + och * 128:k16 * K + och * 128 + 128]
                        for tch in range(2):
                            nc.tensor.matmul(
                                cps[och][tch][:], lhsT,
                                kt_cur[:, k + tch * 512:k + tch * 512 + 512],
                                start=(i == 0 and k == 0),
                                stop=(i == IT - 1 and k == KER - 1))
            kt_cur = kt_next

        p_keys.release()
        p_slab.release()
        ps_k.release()
        # take the freed kps banks (WAR-safe: conv matmuls are their last
        # readers) rather than the conv banks still being drained below
        ps_q = tc.alloc_tile_pool(name="qps", bufs=2, space="PSUM")

        # ---- P3a: drain conv PSUM and launch the pairwise AllReduce ------
        # (emitted BEFORE the q-projection so the collective overlaps it;
        # no PE instructions in this block)
        p_qT = tc.alloc_tile_pool(name="qT", bufs=1, side="right")
        p_kcf = tc.alloc_tile_pool(name="kcf", bufs=1, side="right")
        p_kc = tc.alloc_tile_pool(name="kc", bufs=2, side="right")
        p_dram = tc.alloc_tile_pool(name="cc", bufs=1, space="DRAM")

        cc_in = p_dram.tile([2, 128, D], FP16, tag="ccin")
        cc_out = p_dram.tile([2, 128, D], FP16, tag="ccout")
        for och in range(2):
            kcp = p_kc.tile([128, D], FP16, tag="kcio")
            for tch in range(2):
                nc.scalar.activation(kcp[:, tch * 512:(tch + 1) * 512],
                                     cps[och][tch][:], ACTF.Copy)
            nc.sync.dma_start(out=cc_in[och], in_=kcp[:])
        if single_core:
            nc.sync.dma_start(out=cc_out[:], in_=cc_in[:])
        else:
            nc.gpsimd.collective_compute(
                "AllReduce", mybir.AluOpType.add,
                replica_groups=[[0, 1], [2, 3], [4, 5], [6, 7]],
                ins=[cc_in[:]], outs=[cc_out[:]])
        # conv result + bias, in fp32r for the psa/pss matmul operands
        kc_b = p_kcf.tile([128, 2, D], FP32R, tag="kcb")
        for och in range(2):
            kcs = p_kc.tile([128, D], FP16, tag="kcio")
            nc.sync.dma_start(out=kcs[:], in_=cc_out[och])
            nc.vector.tensor_scalar_add(kc_b[:, och, :], kcs[:],
                                        bpk_t[:, och:och + 1])

        # ---- P2: qT = Wq @ x^T  (t on partitions, n free; overlaps the
        # collective above) -----------------------------------------------
        qT_s = p_qT.tile([128, 8, NL], FP16, tag="qT")
        for tt in range(8):
            for nch in range(4):
                psq = ps_q.tile([128, 512], FP32, tag="psq")
                for a in range(8):
                    nc.tensor.matmul(
                        psq[:], wqT_s[:, a, tt * 128:(tt + 1) * 128],
                        xT_s[:, nch, a, :],
                        start=(a == 0), stop=(a == 7))
                nc.scalar.activation(qT_s[:, tt, nch * 512:(nch + 1) * 512],
                                     psq[:], ACTF.Copy)
        ps_q.release()
        ps_conv.release()
        p_x.release()
        p_w2.release()
        p_w.release()

        # ---- P3b: transpose conv output for the dots lhsT ----------------
        ps_t = tc.alloc_tile_pool(name="tps", bufs=2, space="PSUM")
        kcT = p_kcf.tile([128, 8, K], FP16, tag="kcT")
        for tt in range(8):
            pst = ps_t.tile([128, K], FP32R, tag="pst")
            nc.tensor.transpose(pst[:, 0:128],
                                kc_b[:, 0, tt * 128:(tt + 1) * 128], ident[:])
            nc.tensor.transpose(pst[:, 128:256],
                                kc_b[:, 1, tt * 128:(tt + 1) * 128], ident[:])
            nc.scalar.activation(kcT[:, tt, :], pst[:], ACTF.Copy)
        p_kc.release()
        ps_t.release()

        # ---- P4+P5: attention with the output projection interleaved -----
        # P5 tiles for a finished n-chunk are emitted between the next
        # chunk's attention iterations, keeping the PE fed while the
        # exp->sum->normalize chains resolve (and keeping HAM warm).
        p_attn = tc.alloc_tile_pool(name="attnT", bufs=1, side="left")
        p_y = tc.alloc_tile_pool(name="ysb", bufs=3, side="right")
        p_exp = tc.alloc_tile_pool(name="exp", bufs=6, side="right")
        p_rec = tc.alloc_tile_pool(name="rec", bufs=3, side="right")
        p_rec = tc.alloc_tile_pool(name="rec", bufs=3, side="right")
        # 256-wide iterations, manually packed into full PSUM banks:
        # one [128,512] bank per iteration's psd lo+hi (4 banks = depth 4),
        # one bank holding two iterations' pss halves, one for psa halves,
        # two banks for the interleaved output projection -> 8 total
        ps_d = tc.alloc_tile_pool(name="dps", bufs=4, space="PSUM")
        ps_s = tc.alloc_tile_pool(name="sps", bufs=1, space="PSUM")
        ps_a = tc.alloc_tile_pool(name="aps", bufs=1, space="PSUM")
        ps_y = tc.alloc_tile_pool(name="yps", bufs=2, space="PSUM")

        psd_banks = [ps_d.tile([128, 512], FP32, tag="psd", name=f"psdb{i}")
                     for i in range(4)]
        pss_bank = ps_s.tile([128, 512], FP32, tag="pss", name="pssb")
        psa_bank = ps_a.tile([128, 512], FP32, tag="psa", name="psab")

        attn_outT = p_attn.tile([128, 8, NL], FP16, tag="attnT")

        def emit_dots(j, nch, h):
            # 512-wide: 213ns matmuls leave slack for the 104ns LDWEIGHTS
            # to hide (256-wide left none); psd depth 2 across the four
            # banks, interleaved y-tiles provide the chain slack
            nsl = slice(nch * 512, (nch + 1) * 512)
            psd_lo = psd_banks[(2 * j) % 4]
            nc.tensor.matmul(psd_lo[:], kcT[:, h, 0:128], qT_s[:, h, nsl],
                             start=True, stop=True)
            psd_hi = psd_banks[(2 * j + 1) % 4]
            nc.tensor.matmul(psd_hi[:], kcT[:, h, 128:256], qT_s[:, h, nsl],
                             start=True, stop=True)
            e_lo = p_exp.tile([128, 512], FP32R, tag="exp", name=f"el{nch}_{h}")
            nc.scalar.activation(e_lo[:], psd_lo[:], ACTF.Exp, scale=SCALE)
            e_hi = p_exp.tile([128, 512], FP32R, tag="exp", name=f"eh{nch}_{h}")
            nc.scalar.activation(e_hi[:], psd_hi[:], ACTF.Exp, scale=SCALE)
            return nsl, e_lo, e_hi

        def emit_tail(j, nch, h, st):
            nsl, e_lo, e_hi = st
            # all-ones stationary -> every PSUM partition holds the K-sum:
            # the normalizer arrives pre-broadcast
            pss = pss_bank[:]
            nc.tensor.matmul(pss, ones[:], e_lo[:], start=True, stop=False)
            nc.tensor.matmul(pss, ones[:], e_hi[:], start=False, stop=True)
            psa = psa_bank[:]
            nc.tensor.matmul(psa, kc_b[:, 0, h * 128:(h + 1) * 128],
                             e_lo[:], start=True, stop=False)
            nc.tensor.matmul(psa, kc_b[:, 1, h * 128:(h + 1) * 128],
                             e_hi[:], start=False, stop=True)
            rec = p_rec.tile([128, 512], FP32, tag="rec", name=f"rec{nch}_{h}")
            nc.vector.reciprocal_approx_fast(rec[:], pss)
            nc.vector.tensor_mul(attn_outT[:, h, nsl], psa, rec[:])

        def emit_y_tile(nt):
            psy = [ps_y.tile([128, 512], FP32, tag="psy", name=f"psy{nt}_{_i}")
                   for _i in range(2)]
            for tt in range(8):
                for cch in range(2):
                    nc.tensor.matmul(
                        psy[cch][:], attn_outT[:, tt, nt * 128:(nt + 1) * 128],
                        woT_s[:, tt, cch * 512:(cch + 1) * 512],
                        start=(tt == 0), stop=(tt == 7))
            ysb = p_y.tile([128, D], FP16, tag="ysb")
            for cch in range(2):
                nc.vector.tensor_add(ysb[:, cch * 512:(cch + 1) * 512],
                                     psy[cch][:],
                                     bo_bc[:, cch * 512:(cch + 1) * 512])
            DQ[nt % 2].dma_start(out=y_out.ap()[nt * 128:(nt + 1) * 128, :],
                                 in_=ysb[:])

        seq = [(nch, h) for nch in range(4) for h in range(8)]
        pending = []          # y tiles whose attention chunk is complete
        st = emit_dots(0, *seq[0])
        for j, (nch, h) in enumerate(seq):
            nxt = emit_dots(j + 1, *seq[j + 1]) if j + 1 < len(seq) else None
            emit_tail(j, nch, h, st)
            st = nxt
            if h % 2 == 1 and pending:
                emit_y_tile(pending.pop(0))
            if h == 7:
                pending += [nch * 4 + t for t in range(4)]
        for nt in pending:
            emit_y_tile(nt)
        p_rec.release()
        p_rec.release()
        p_exp.release()
        p_y.release()
        p_kcf.release()
        p_qT.release()
        ps_y.release()
        ps_a.release()
        ps_s.release()
        ps_d.release()
        p_attn.release()
        p_wo.release()
        consts.release()

    nc.compile()
    return nc


def _get_nc():
    if "nc" not in _CACHE:
        _CACHE["nc"] = _build()
    return _CACHE["nc"]


def _swiz(wT):
    # [D, cols] -> [128, 8, cols]: partition-major blocks of 128 rows
    return np.ascontiguousarray(
        wT.reshape(8, 128, wT.shape[1]).transpose(1, 0, 2))


def _prep_inputs(x, Wq, Wk, Wpk, bpk, Wo, bo):
    wqT = _swiz(Wq.T.astype(np.float16))
    wkT = _swiz(Wk.T.astype(np.float16))
    woT = _swiz(Wo.T.astype(np.float16))
    # Wpk (K=256, N=4096, KER=32) -> [i, k, o] contiguous
    wpkT = np.ascontiguousarray(
        Wpk.astype(np.float16).transpose(1, 2, 0)).reshape(N, KER * K)
    bpk2 = np.ascontiguousarray(bpk.astype(np.float32).reshape(2, 128).T)
    bo2 = np.ascontiguousarray(bo.astype(np.float32).reshape(1, D))
    ones = np.ones((128, 128), dtype=np.float32)
    ident = np.eye(128, dtype=np.float32)
    in_maps = []
    for c in range(NCORES):
        b, s = c // 2, c % 2
        in_maps.append({
            "xT": np.ascontiguousarray(
                x[b, s * NL:(s + 1) * NL, :].T.astype(np.float16)
                .reshape(8, 128, 4, 512).transpose(1, 2, 0, 3)),
            "wqT": wqT, "wkT": wkT, "woT": woT,
            "wpkT": np.ascontiguousarray(wpkT[s * NL:(s + 1) * NL]),
            "bpk": bpk2, "bo": bo2, "ones": ones, "ident": ident,
        })
    return in_maps


def kernel(x, Wq, Wk, Wpk, bpk, Wo, bo, _trace=False, _trace_kwargs=None):
    from concourse.bass_utils import run_bass_kernel_spmd

    nc = _get_nc()
    in_maps = _prep_inputs(np.asarray(x), np.asarray(Wq), np.asarray(Wk),
                           np.asarray(Wpk), np.asarray(bpk), np.asarray(Wo),
                           np.asarray(bo))
    res = run_bass_kernel_spmd(nc, in_maps, core_ids=list(range(NCORES)),
                               trace=_trace, **(_trace_kwargs or {}))
    _CACHE["last_result"] = res
    out = np.empty((B, N, D), dtype=np.float32)
    for c in range(NCORES):
        b, s = c // 2, c % 2
        out[b, s * NL:(s + 1) * NL, :] = res.results[c]["y"].astype(np.float32)
    return out


# revision 64
# speedup vs baseline: 1.1615x; 1.0162x over previous
"""ConvLinformer self-attention on 8 Trainium2 NeuronCores.

Sharding: 8 cores = (batch b, sequence-half s); B=4, N=4096 -> each core owns
2048 sequence rows of one batch. The conv (the dominant 275 GFLOP op) contracts
over the sequence dim, so each core computes a partial conv over its own rows
using only its half of the conv weight (host pre-transposed to [i, k, o] layout
for clean DMA + matmul lhsT tiles); a pairwise AllReduce of the small (256,1024)
conv output completes it. Attention (all 8 heads, own rows) then needs no
further communication, and neither does the output projection.

v2: fp16 operands for all matmuls (PSUM stays fp32), AllReduce overlapped
behind the q-projection, tighter startup prefetch, and a softmax tail that
row-sums via an all-ones 128x128 stationary operand so the sums land
pre-broadcast in PSUM (vector reciprocal + multiply, no serial [1,512]
reciprocal, no gpsimd broadcast on the critical path).
"""

import sys

sys.path.insert(0, "/opt/trn_rl_repo")

import numpy as np

B, N, D = 4, 4096, 1024
H, DH, K = 8, 128, 256
KER = 32
PADL = 15
NL = N // 2          # rows per core
NCORES = 8
SCALE = DH ** -0.5

_CACHE = {}


def _build(single_core=False):
    import concourse.bacc as bacc
    import concourse.mybir as mybir
    import concourse.tile as tile

    FP32 = mybir.dt.float32
    FP32R = mybir.dt.float32r
    FP16 = mybir.dt.float16
    ACTF = mybir.ActivationFunctionType

    nc = bacc.Bacc("TRN2", target_bir_lowering=False, debug=False,
                   num_devices=1 if single_core else NCORES)

    # host pre-swizzled to [128 partitions, 8 blocks, cols] so each SBUF
    # tile loads with a single contiguous DMA (DMA triggers cost ~600ns
    # of queue time each)
    # chunk-major x: [part, 512-col chunk, block, col] so each chunk is one
    # contiguous DMA on both sides and the first chunk feeds keys tiles 0-3
    xT = nc.dram_tensor("xT", (128, 4, 8, 512), FP16, kind="ExternalInput")
    wqT = nc.dram_tensor("wqT", (128, 8, D), FP16, kind="ExternalInput")
    wkT = nc.dram_tensor("wkT", (128, 2, 8, 512), FP16, kind="ExternalInput")
    woT = nc.dram_tensor("woT", (128, 8, D), FP16, kind="ExternalInput")
    wpkT = nc.dram_tensor("wpkT", (NL, KER * K), FP16, kind="ExternalInput")
    bpk_in = nc.dram_tensor("bpk", (128, 2), FP32, kind="ExternalInput")
    bo_in = nc.dram_tensor("bo", (1, D), FP32, kind="ExternalInput")
    ones_in = nc.dram_tensor("ones", (128, 128), FP32R, kind="ExternalInput")
    ident_in = nc.dram_tensor("ident", (128, 128), FP32R, kind="ExternalInput")
    y_out = nc.dram_tensor("y", (NL, D), FP16, kind="ExternalOutput")

    IT = NL // 128        # 16 i-tiles
    TPAD = 1056           # padded conv spatial width (15 + 1024 + 17)

    with tile.TileContext(nc) as tc:
        # ---- long-lived pools -------------------------------------------
        consts = tc.alloc_tile_pool(name="consts", bufs=1, side="left")
        p_wo = tc.alloc_tile_pool(name="wo", bufs=1, side="left")
        p_w = tc.alloc_tile_pool(name="w", bufs=1, side="left")
        p_w2 = tc.alloc_tile_pool(name="w2", bufs=1, side="left")
        p_x = tc.alloc_tile_pool(name="x", bufs=1, side="left")
        p_slab = tc.alloc_tile_pool(name="slab", bufs=4, side="left")
        p_keys = tc.alloc_tile_pool(name="keys", bufs=3, side="left")
        ps_conv = tc.alloc_tile_pool(name="convps", bufs=1, space="PSUM")
        ps_k = tc.alloc_tile_pool(name="kps", bufs=2, space="PSUM")

        # critical-path DMAs first: the first keys matmul needs wk[a] + the
        # leading x columns; interleave a-wise so matmul a=0 can start early
        # two HW DGE queues exist (SP + Activation): alternate bulk DMAs
        # across them so independent transfers run in parallel
        DQ = [nc.sync, nc.scalar]
        wkT_s = p_w.tile([128, 2, 8, 512], FP16, tag="wk")
        xT_s = p_x.tile([128, 4, 8, 512], FP16, tag="xT")

        def emit_slab_dma(i, kh):
            slab = p_slab.tile([128, 16 * K], FP16, tag="slab")
            DQ[kh].dma_start(
                out=slab[:],
                in_=wpkT.ap()[i * 128:(i + 1) * 128,
                              kh * 16 * K:(kh + 1) * 16 * K])
            return slab

        # contiguous per-block chunks, balanced across both DGE queues in
        # keys-consumption order (a=0..7 with the matching wk pair just
        # ahead); slab0 lands mid-stream so conv i=0 is never slab-bound
        ones = consts.tile([128, 128], FP32R, tag="ones")
        nc.sync.dma_start(out=ones[:], in_=ones_in.ap())
        nc.sync.dma_start(out=xT_s[:, 0], in_=xT.ap()[:, 0])
        nc.scalar.dma_start(out=wkT_s[:, 0], in_=wkT.ap()[:, 0])
        nc.scalar.dma_start(out=wkT_s[:, 1], in_=wkT.ap()[:, 1])
        slabs = [emit_slab_dma(0, 0), emit_slab_dma(0, 1)]
        nc.sync.dma_start(out=xT_s[:, 1], in_=xT.ap()[:, 1])
        nc.scalar.dma_start(out=xT_s[:, 2], in_=xT.ap()[:, 2])
        nc.sync.dma_start(out=xT_s[:, 3], in_=xT.ap()[:, 3])

        ident = consts.tile([128, 128], FP32R, tag="ident")
        nc.sync.dma_start(out=ident[:], in_=ident_in.ap())
        bpk_t = consts.tile([128, 2], FP32, tag="bpk")
        nc.sync.dma_start(out=bpk_t[:], in_=bpk_in.ap())
        bo_row = consts.tile([1, D], FP32, tag="borow")
        nc.sync.dma_start(out=bo_row[:], in_=bo_in.ap())
        bo_bc = consts.tile([128, D], FP32, tag="bobc")
        nc.gpsimd.partition_broadcast(bo_bc[:], bo_row[:])

        # ---- P1: keys production + conv accumulation --------------------
        cps = [[ps_conv.tile([128, 512], FP32, tag=f"cps{o}{t}", name=f"cps{o}{t}")
                for t in range(2)] for o in range(2)]

        def emit_keys(i):
            pks = []
            for tch in range(2):
                psk = ps_k.tile([128, 512], FP32, tag="psk")
                for a in range(8):
                    nc.tensor.matmul(
                        psk[:], xT_s[:, i // 4, a,
                                     (i % 4) * 128:(i % 4) * 128 + 128],
                        wkT_s[:, tch, a, :],
                        start=(a == 0), stop=(a == 7))
                pks.append(psk)
            kt = p_keys.tile([128, TPAD], FP16, tag="keys")
            # fp16 memset via Copy(in*0.0)
            nc.scalar.activation(kt[:, 0:PADL], pks[0][:, 0:PADL],
                                 ACTF.Copy, scale=0.0)
            nc.scalar.activation(kt[:, PADL + D:TPAD],
                                 pks[1][:, 0:TPAD - PADL - D],
                                 ACTF.Copy, scale=0.0)
            nc.scalar.activation(kt[:, PADL:PADL + 512], pks[0][:], ACTF.Copy)
            nc.scalar.activation(kt[:, PADL + 512:PADL + D], pks[1][:], ACTF.Copy)
            return kt

        # prefetch tiles for later weights; DMAs are spread through P1 while
        # the DMA engines are otherwise idle
        wqT_s = p_w2.tile([128, 8, D], FP16, tag="wq")
        woT_s = p_wo.tile([128, 8, D], FP16, tag="wo")

        kt_cur = emit_keys(0)
        for i in range(IT):
            kt_next = emit_keys(i + 1) if i + 1 < IT else None
            if i + 1 < IT:
                slabs += [emit_slab_dma(i + 1, 0), emit_slab_dma(i + 1, 1)]
            if i == 1:
                nc.sync.dma_start(out=wqT_s[:], in_=wqT.ap())
            if i == 3:
                nc.sync.dma_start(out=woT_s[:], in_=woT.ap())
            for kh in range(2):
                slab = slabs.pop(0)
                for k16 in range(16):
                    k = kh * 16 + k16
                    for och in range(2):
                        lhsT = slab[:, k16 * K + och * 128:k16 * K + och * 128 + 128]
                        for tch in range(2):
                            nc.tensor.matmul(
                                cps[och][tch][:], lhsT,
                                kt_cur[:, k + tch * 512:k + tch * 512 + 512],
                                start=(i == 0 and k == 0),
                                stop=(i == IT - 1 and k == KER - 1))
            kt_cur = kt_next

        p_keys.release()
        p_slab.release()
        ps_k.release()
        # take the freed kps banks (WAR-safe: conv matmuls are their last
        # readers) rather than the conv banks still being drained below
        ps_q = tc.alloc_tile_pool(name="qps", bufs=2, space="PSUM")

        # ---- P3a: drain conv PSUM and launch the pairwise AllReduce ------
        # (emitted BEFORE the q-projection so the collective overlaps it;
        # no PE instructions in this block)
        p_qT = tc.alloc_tile_pool(name="qT", bufs=1, side="right")
        p_kcf = tc.alloc_tile_pool(name="kcf", bufs=1, side="right")
        p_kc = tc.alloc_tile_pool(name="kc", bufs=2, side="right")
        p_dram = tc.alloc_tile_pool(name="cc", bufs=1, space="DRAM")

        cc_in = p_dram.tile([2, 128, D], FP16, tag="ccin")
        cc_out = p_dram.tile([2, 128, D], FP16, tag="ccout")
        for och in range(2):
            kcp = p_kc.tile([128, D], FP16, tag="kcio")
            for tch in range(2):
                nc.scalar.activation(kcp[:, tch * 512:(tch + 1) * 512],
                                     cps[och][tch][:], ACTF.Copy)
            nc.sync.dma_start(out=cc_in[och], in_=kcp[:])
        if single_core:
            nc.sync.dma_start(out=cc_out[:], in_=cc_in[:])
        else:
            nc.gpsimd.collective_compute(
                "AllReduce", mybir.AluOpType.add,
                replica_groups=[[0, 1], [2, 3], [4, 5], [6, 7]],
                ins=[cc_in[:]], outs=[cc_out[:]])
        # conv result + bias, in fp32r for the psa/pss matmul operands
        kc_b = p_kcf.tile([128, 2, D], FP32R, tag="kcb")
        for och in range(2):
            kcs = p_kc.tile([128, D], FP16, tag="kcio")
            nc.sync.dma_start(out=kcs[:], in_=cc_out[och])
            nc.vector.tensor_scalar_add(kc_b[:, och, :], kcs[:],
                                        bpk_t[:, och:och + 1])

        # ---- P2: qT = Wq @ x^T  (t on partitions, n free; overlaps the
        # collective above) -----------------------------------------------
        qT_s = p_qT.tile([128, 8, NL], FP16, tag="qT")
        for tt in range(8):
            for nch in range(4):
                psq = ps_q.tile([128, 512], FP32, tag="psq")
                for a in range(8):
                    nc.tensor.matmul(
                        psq[:], wqT_s[:, a, tt * 128:(tt + 1) * 128],
                        xT_s[:, nch, a, :],
                        start=(a == 0), stop=(a == 7))
                nc.scalar.activation(qT_s[:, tt, nch * 512:(nch + 1) * 512],
                                     psq[:], ACTF.Copy)
        ps_q.release()
        ps_conv.release()
        p_x.release()
        p_w2.release()
        p_w.release()

        # ---- P3b: transpose conv output for the dots lhsT ----------------
        ps_t = tc.alloc_tile_pool(name="tps", bufs=2, space="PSUM")
        kcT = p_kcf.tile([128, 8, K], FP16, tag="kcT")
        for tt in range(8):
            pst = ps_t.tile([128, K], FP32R, tag="pst")
            nc.tensor.transpose(pst[:, 0:128],
                                kc_b[:, 0, tt * 128:(tt + 1) * 128], ident[:])
            nc.tensor.transpose(pst[:, 128:256],
                                kc_b[:, 1, tt * 128:(tt + 1) * 128], ident[:])
            nc.scalar.activation(kcT[:, tt, :], pst[:], ACTF.Copy)
        p_kc.release()
        ps_t.release()

        # ---- P4+P5: attention with the output projection interleaved -----
        # P5 tiles for a finished n-chunk are emitted between the next
        # chunk's attention iterations, keeping the PE fed while the
        # exp->sum->normalize chains resolve (and keeping HAM warm).
        p_attn = tc.alloc_tile_pool(name="attnT", bufs=1, side="left")
        p_y = tc.alloc_tile_pool(name="ysb", bufs=3, side="right")
        p_exp = tc.alloc_tile_pool(name="exp", bufs=6, side="right")
        p_rec = tc.alloc_tile_pool(name="rec", bufs=3, side="right")
        p_rec = tc.alloc_tile_pool(name="rec", bufs=3, side="right")
        # 256-wide iterations, manually packed into full PSUM banks:
        # one [128,512] bank per iteration's psd lo+hi (4 banks = depth 4),
        # one bank holding two iterations' pss halves, one for psa halves,
        # two banks for the interleaved output projection -> 8 total
        ps_d = tc.alloc_tile_pool(name="dps", bufs=4, space="PSUM")
        ps_s = tc.alloc_tile_pool(name="sps", bufs=1, space="PSUM")
        ps_a = tc.alloc_tile_pool(name="aps", bufs=1, space="PSUM")
        ps_y = tc.alloc_tile_pool(name="yps", bufs=2, space="PSUM")

        psd_banks = [ps_d.tile([128, 512], FP32, tag="psd", name=f"psdb{i}")
                     for i in range(4)]
        pss_bank = ps_s.tile([128, 512], FP32, tag="pss", name="pssb")
        psa_bank = ps_a.tile([128, 512], FP32, tag="psa", name="psab")

        attn_outT = p_attn.tile([128, 8, NL], FP16, tag="attnT")

        def emit_dots(j, nch, h):
            # 512-wide: 213ns matmuls leave slack for the 104ns LDWEIGHTS
            # to hide (256-wide left none); psd depth 2 across the four
            # banks, interleaved y-tiles provide the chain slack
            nsl = slice(nch * 512, (nch + 1) * 512)
            psd_lo = psd_banks[(2 * j) % 4]
            nc.tensor.matmul(psd_lo[:], kcT[:, h, 0:128], qT_s[:, h, nsl],
                             start=True, stop=True)
            psd_hi = psd_banks[(2 * j + 1) % 4]
            nc.tensor.matmul(psd_hi[:], kcT[:, h, 128:256], qT_s[:, h, nsl],
                             start=True, stop=True)
            e_lo = p_exp.tile([128, 512], FP32R, tag="exp", name=f"el{nch}_{h}")
            nc.scalar.activation(e_lo[:], psd_lo[:], ACTF.Exp, scale=SCALE)
            e_hi = p_exp.tile([128, 512], FP32R, tag="exp", name=f"eh{nch}_{h}")
            nc.scalar.activation(e_hi[:], psd_hi[:], ACTF.Exp, scale=SCALE)
            return nsl, e_lo, e_hi

        def emit_tail(j, nch, h, st):
            nsl, e_lo, e_hi = st
            # all-ones stationary -> every PSUM partition holds the K-sum:
            # the normalizer arrives pre-broadcast
            pss = pss_bank[:]
            nc.tensor.matmul(pss, ones[:], e_lo[:], start=True, stop=False)
            nc.tensor.matmul(pss, ones[:], e_hi[:], start=False, stop=True)
            psa = psa_bank[:]
            nc.tensor.matmul(psa, kc_b[:, 0, h * 128:(h + 1) * 128],
                             e_lo[:], start=True, stop=False)
            nc.tensor.matmul(psa, kc_b[:, 1, h * 128:(h + 1) * 128],
                             e_hi[:], start=False, stop=True)
            rec = p_rec.tile([128, 512], FP32, tag="rec", name=f"rec{nch}_{h}")
            nc.vector.reciprocal_approx_fast(rec[:], pss)
            nc.vector.tensor_mul(attn_outT[:, h, nsl], psa, rec[:])

        def emit_y_tile(nt):
            psy = [ps_y.tile([128, 512], FP32, tag="psy", name=f"psy{nt}_{_i}")
                   for _i in range(2)]
            for tt in range(8):
                for cch in range(2):
                    nc.tensor.matmul(
                        psy[cch][:], attn_outT[:, tt, nt * 128:(nt + 1) * 128],
                        woT_s[:, tt, cch * 512:(cch + 1) * 512],
                        start=(tt == 0), stop=(tt == 7))
            ysb = p_y.tile([128, D], FP16, tag="ysb")
            for cch in range(2):
                nc.vector.tensor_add(ysb[:, cch * 512:(cch + 1) * 512],
                                     psy[cch][:],
                                     bo_bc[:, cch * 512:(cch + 1) * 512])
            DQ[nt % 2].dma_start(out=y_out.ap()[nt * 128:(nt + 1) * 128, :],
                                 in_=ysb[:])

        seq = [(nch, h) for nch in range(4) for h in range(8)]
        pending = []          # y tiles whose attention chunk is complete
        st = emit_dots(0, *seq[0])
        for j, (nch, h) in enumerate(seq):
            nxt = emit_dots(j + 1, *seq[j + 1]) if j + 1 < len(seq) else None
            emit_tail(j, nch, h, st)
            st = nxt
            if h % 2 == 1 and pending:
                emit_y_tile(pending.pop(0))
            if h == 7:
                pending += [nch * 4 + t for t in range(4)]
        for nt in pending:
            emit_y_tile(nt)
        p_rec.release()
        p_rec.release()
        p_exp.release()
        p_y.release()
        p_kcf.release()
        p_qT.release()
        ps_y.release()
        ps_a.release()
        ps_s.release()
        ps_d.release()
        p_attn.release()
        p_wo.release()
        consts.release()

    nc.compile()
    return nc


def _get_nc():
    if "nc" not in _CACHE:
        _CACHE["nc"] = _build()
    return _CACHE["nc"]


def _swiz(wT):
    # [D, cols] -> [128, 8, cols]: partition-major blocks of 128 rows
    return np.ascontiguousarray(
        wT.reshape(8, 128, wT.shape[1]).transpose(1, 0, 2))


def _prep_inputs(x, Wq, Wk, Wpk, bpk, Wo, bo):
    wqT = _swiz(Wq.T.astype(np.float16))
    wkT = np.ascontiguousarray(
        Wk.T.astype(np.float16).reshape(8, 128, 2, 512).transpose(1, 2, 0, 3))
    woT = _swiz(Wo.T.astype(np.float16))
    # Wpk (K=256, N=4096, KER=32) -> [i, k, o] contiguous
    wpkT = np.ascontiguousarray(
        Wpk.astype(np.float16).transpose(1, 2, 0)).reshape(N, KER * K)
    bpk2 = np.ascontiguousarray(bpk.astype(np.float32).reshape(2, 128).T)
    bo2 = np.ascontiguousarray(bo.astype(np.float32).reshape(1, D))
    ones = np.ones((128, 128), dtype=np.float32)
    ident = np.eye(128, dtype=np.float32)
    in_maps = []
    for c in range(NCORES):
        b, s = c // 2, c % 2
        in_maps.append({
            "xT": np.ascontiguousarray(
                x[b, s * NL:(s + 1) * NL, :].T.astype(np.float16)
                .reshape(8, 128, 4, 512).transpose(1, 2, 0, 3)),
            "wqT": wqT, "wkT": wkT, "woT": woT,
            "wpkT": np.ascontiguousarray(wpkT[s * NL:(s + 1) * NL]),
            "bpk": bpk2, "bo": bo2, "ones": ones, "ident": ident,
        })
    return in_maps


def kernel(x, Wq, Wk, Wpk, bpk, Wo, bo, _trace=False, _trace_kwargs=None):
    from concourse.bass_utils import run_bass_kernel_spmd

    nc = _get_nc()
    in_maps = _prep_inputs(np.asarray(x), np.asarray(Wq), np.asarray(Wk),
                           np.asarray(Wpk), np.asarray(bpk), np.asarray(Wo),
                           np.asarray(bo))
    res = run_bass_kernel_spmd(nc, in_maps, core_ids=list(range(NCORES)),
                               trace=_trace, **(_trace_kwargs or {}))
    _CACHE["last_result"] = res
    out = np.empty((B, N, D), dtype=np.float32)
    for c in range(NCORES):
        b, s = c // 2, c % 2
        out[b, s * NL:(s + 1) * NL, :] = res.results[c]["y"].astype(np.float32)
    return out
